# revision 25
# baseline (speedup 1.0000x reference)
"""Multi-head causal attention with RoPE on 8 Trainium2 NeuronCores.

Sharding: data-parallel over batch (2 groups of 4 cores) x tensor-parallel
over heads (4 heads / 512 cols of Wq/Wk/Wv per core, 512 rows of Wo).
Each core computes its head-group's Q/K/V projections in transposed layout
([head_dim, seq] -- so no on-device transposes are ever needed), applies
RoPE, runs causal softmax attention (scores kept transposed [tk, tq];
row sums via a ones-vector matmul), and emits its partial output
projection.  The host sums the 4 partials per batch element.

v2 pipeline structure (PE-bound; per-chunk software pipeline):
  for c: [dma c+1] proj K/V/Q(c) -> attn(c) interleaved with outproj(c-1)
  - row-sum (l) matmuls for a head pair packed into PE column groups 0/64
  - softmax normalization chain (ACT Reciprocal -> broadcast matmul ->
    copy/mul on DVE) software-pipelined so it never blocks the PE
  - all PSUM evictions on VectorE; ScalarE only does exp / rope-copy / recip
  - outputs stored bf16, summed on host in fp32

Self-contained: shapes/sharding hardcoded for
  q_input/kv_input [2, 2048, 2048], 16 heads x 128 head_dim.
"""

import math

import numpy as np
import ml_dtypes

B, T, D, H = 2, 2048, 2048, 16
HD = 128          # head dim
HALF = HD // 2    # rope half
P = 128           # partitions
CHUNK = 512       # tq / free-dim chunk
NCORES = 8
GROUPS = 4        # head-groups (tensor-parallel degree per batch)
HPG = H // GROUPS # heads per group
GD = HPG * HD     # group width (512)
DT = D // P       # d-tiles (16)
DG = 4            # d-tiles per DMA group
NDG = DT // DG    # DMA groups (4)
TCH = T // CHUNK  # seq chunks (4)
TKT = T // P      # tk tiles (16)
OCH = 512         # outproj free-dim chunk

TRACE = False       # set True before calling kernel() to capture an NTFF trace
LAST_RESULT = None  # BassKernelResults of the last kernel() call
PACK_L = True       # pack the two row-sum matmuls of a head pair via col tiling

_cache = {}


def _build_program(actions, npat):
    """Build the per-core Bass program.

    actions: {(c, t): "full" | pattern_index} for every (tq-chunk, tk-tile)
    score block that has at least one unmasked element.
    """
    from contextlib import ExitStack

    import concourse.mybir as mybir
    import concourse.tile as tile
    from concourse import bacc
    from concourse.bass import ts

    fp32 = mybir.dt.float32
    bf16 = mybir.dt.bfloat16
    Copy = mybir.ActivationFunctionType.Copy
    Exp = mybir.ActivationFunctionType.Exp
    SCALE = 1.0 / math.sqrt(HD)

    nc = bacc.Bacc(
        "TRN2",
        target_bir_lowering=False,
        debug=False,
        enable_asserts=False,
        num_devices=NCORES,
    )

    xqT = nc.dram_tensor("xqT", [D, T], bf16, kind="ExternalInput").ap()
    xkvT = nc.dram_tensor("xkvT", [D, T], bf16, kind="ExternalInput").ap()
    wq = nc.dram_tensor("wq", [D, GD], bf16, kind="ExternalInput").ap()
    wk = nc.dram_tensor("wk", [D, GD], bf16, kind="ExternalInput").ap()
    wv = nc.dram_tensor("wv", [D, GD], bf16, kind="ExternalInput").ap()
    wo = nc.dram_tensor("wo", [GD, D], bf16, kind="ExternalInput").ap()
    # RoPE in head-dim-interleaved space (host permutes Wq/Wk columns so the
    # rope pair (j, j+64) lands on adjacent partitions (2j, 2j+1); scores are
    # invariant to a common Q/K head-dim permutation):
    #   rope'(x) = x * cs2 + swap_adjacent_pairs(x) * ss2
    # cs2[2j] = cs2[2j+1] = cos_j ; ss2[2j] = -sin_j, ss2[2j+1] = +sin_j
    cs2 = nc.dram_tensor("cs2", [P, T], bf16, kind="ExternalInput").ap()
    ss2 = nc.dram_tensor("ss2", [P, T], bf16, kind="ExternalInput").ap()
    pat = nc.dram_tensor("pat", [npat, P, CHUNK], bf16, kind="ExternalInput").ap()
    out = nc.dram_tensor("out", [T, D], bf16, kind="ExternalOutput").ap()

    xqT_r = xqT.rearrange("(dt p) t -> p dt t", p=P)
    xkvT_r = xkvT.rearrange("(dt p) t -> p dt t", p=P)

    with ExitStack() as ctx:
        tc = ctx.enter_context(tile.TileContext(nc))
        const_pool = ctx.enter_context(tc.tile_pool(name="const", bufs=1))
        xk_pool = ctx.enter_context(tc.tile_pool(name="xk", bufs=6))
        xq_pool = ctx.enter_context(tc.tile_pool(name="xq", bufs=6))
        rope_pool = ctx.enter_context(tc.tile_pool(name="rope", bufs=2))
        exp_pool = ctx.enter_context(tc.tile_pool(name="exp", bufs=8))
        osb_pool = ctx.enter_context(tc.tile_pool(name="osb", bufs=3))
        lb_pool = ctx.enter_context(tc.tile_pool(name="lb", bufs=2))
        mm_psum = ctx.enter_context(tc.tile_pool(name="mmps", bufs=2, space="PSUM"))
        s_psum = ctx.enter_context(tc.tile_pool(name="sps", bufs=2, space="PSUM"))
        o_psum = ctx.enter_context(tc.tile_pool(name="ops", bufs=2, space="PSUM"))
        l_psum = ctx.enter_context(tc.tile_pool(name="lps", bufs=2, space="PSUM"))

        # persistent SBUF tensors
        wq_sb = const_pool.tile([P, DT, GD], bf16, tag="wq")
        wk_sb = const_pool.tile([P, DT, GD], bf16, tag="wk")
        wv_sb = const_pool.tile([P, DT, GD], bf16, tag="wv")
        wo_sb = const_pool.tile([P, HPG, D], bf16, tag="wo")
        cs2_sb = const_pool.tile([P, T], bf16, tag="cs2")
        ss2_sb = const_pool.tile([P, T], bf16, tag="ss2")
        pat_sb = const_pool.tile([P, npat, CHUNK], bf16, tag="pat")
        # all-ones [128, 128] stationary: the row-sum matmul then emits l
        # broadcast across every output partition at no extra cost (cost ~ N)
        ones_sb = const_pool.tile([P, P], bf16, tag="ones")
        KT = const_pool.tile([P, HPG, T], bf16, tag="KT")
        V = const_pool.tile([P, TKT, GD], bf16, tag="V")
        QT = const_pool.tile([P, HPG, 2, CHUNK], bf16, tag="QT")  # 2-chunk ring
        AT = const_pool.tile([P, HPG, 2, CHUNK], bf16, tag="AT")  # 2-chunk ring

        nc.vector.memset(ones_sb[:], 1.0)

        def dma_w(dst_sb, src, g):
            nc.sync.dma_start(
                dst_sb[:, ts(g, DG), :],
                src.rearrange("(dt p) n -> p dt n", p=P)[:, ts(g, DG), :],
            )

        def dma_x(pool, tag, src_r, c, g):
            t = pool.tile([P, DG, CHUNK], bf16, tag=tag)
            nc.sync.dma_start(t[:], src_r[:, ts(g, DG), ts(c, CHUNK)])
            return t

        # ---- prologue DMAs, ordered so the PE can start ~immediately ----
        xk_t = {}
        for g in range(NDG):
            dma_w(wk_sb, wk, g)
            xk_t[(0, g)] = dma_x(xk_pool, "xk", xkvT_r, 0, g)
        nc.sync.dma_start(cs2_sb[:], cs2)
        nc.sync.dma_start(ss2_sb[:], ss2)
        for g in range(NDG):
            dma_w(wv_sb, wv, g)
        for g in range(NDG):
            dma_w(wq_sb, wq, g)
        xq_t = {(0, g): dma_x(xq_pool, "xq", xqT_r, 0, g) for g in range(NDG)}
        nc.sync.dma_start(pat_sb[:], pat.rearrange("j p n -> p j n"))
        wo_r = wo.rearrange("(h p) n -> p h n", p=P)
        for h in range(HPG):
            nc.sync.dma_start(wo_sb[:, h, :], wo_r[:, h, :])

        SHUF_MASK = [i + 1 - 2 * (i % 2) for i in range(32)]  # [1,0,3,2,...]

        def rope_evict(ps, c, dest_ap):
            # ps: PSUM [P, CHUNK] fp32, partitions = interleaved head_dim
            raw = rope_pool.tile([P, CHUNK], bf16, tag="raw")
            nc.scalar.activation(raw[:], ps[:], Copy)
            rsw = rope_pool.tile([P, CHUNK], bf16, tag="rsw")
            nc.vector.stream_shuffle(rsw[:], raw[:], SHUF_MASK)
            t1 = rope_pool.tile([P, CHUNK], bf16, tag="t1")
            nc.vector.tensor_mul(t1[:], raw[:], cs2_sb[:, ts(c, CHUNK)])
            t2 = rope_pool.tile([P, CHUNK], bf16, tag="t2")
            nc.vector.tensor_mul(t2[:], rsw[:], ss2_sb[:, ts(c, CHUNK)])
            nc.vector.tensor_add(dest_ap, t1[:], t2[:])

        def _proj_psums(douter):
            # d-outer mode holds all four accumulators at once: borrow the
            # (idle at that point) s_psum banks for the upper two
            if douter:
                return [
                    mm_psum.tile([P, CHUNK], fp32, tag="mm", name="pj0"),
                    mm_psum.tile([P, CHUNK], fp32, tag="mm", name="pj1"),
                    s_psum.tile([P, CHUNK], fp32, tag="s", name="pj2"),
                    s_psum.tile([P, CHUNK], fp32, tag="s", name="pj3"),
                ]
            return None

        def proj_qk(c, x_tiles, w_sb, dest, ring, douter=False):
            # douter: issue matmuls d-group-major so the PE starts as soon
            # as the first wk/x d-group DMA lands (chunk-0 warmup)
            pss = _proj_psums(douter)
            order = (
                [(h, d) for d in range(DT) for h in range(HPG)]
                if douter else
                [(h, d) for h in range(HPG) for d in range(DT)]
            )
            for h, d in order:
                if pss is None and d == 0:
                    ps = mm_psum.tile([P, CHUNK], fp32, tag="mm")
                    pss_h = {h: ps}
                if pss is None:
                    ps = pss_h[h]
                else:
                    ps = pss[h]
                nc.tensor.matmul(
                    ps[:], w_sb[:, d, ts(h, HD)], x_tiles[(c, d // DG)][:, d % DG, :],
                    start=(d == 0), stop=(d == DT - 1),
                )
                if d == DT - 1:
                    dest_ap = dest[:, h, c % 2, :] if ring else dest[:, h, ts(c, CHUNK)]
                    rope_evict(ps, c, dest_ap)

        def proj_v(c, x_tiles, douter=False):
            pss = _proj_psums(douter)
            order = (
                [(s, d) for d in range(DT) for s in range(DG)]
                if douter else
                [(s, d) for s in range(DG) for d in range(DT)]
            )
            for s, d in order:
                if pss is None and d == 0:
                    ps = mm_psum.tile([P, GD], fp32, tag="mm")
                    pss_s = {s: ps}
                if pss is None:
                    ps = pss_s[s]
                else:
                    ps = pss[s]
                nc.tensor.matmul(
                    ps[:], x_tiles[(c, d // DG)][:, d % DG, ts(s, P)],
                    wv_sb[:, d, :],
                    start=(d == 0), stop=(d == DT - 1),
                )
                if d == DT - 1:
                    nc.vector.tensor_copy(V[:, c * DG + s, :], ps[:])

        class OutprojEmitter:
            """Emit outproj (m, oc) units in 2-matmul steps so attention
            slots can absorb partial units without over/under-filling."""

            def __init__(self, units):
                self.units = list(units)  # (m, oc) pairs
                self.ps = None
                self.h = 0

            def steps_left(self):
                return (len(self.units) * HPG + (HPG - self.h) % HPG) // 2

            def step(self, nmm=2):
                while nmm > 0:
                    if self.ps is None:
                        if not self.units:
                            return
                        self.m, self.oc = self.units.pop(0)
                        self.ps = mm_psum.tile([P, OCH], fp32, tag="mm", name="opps")
                        self.h = 0
                    c = self.m // DG
                    nc.tensor.matmul(
                        self.ps[:],
                        AT[:, self.h, c % 2, ts(self.m % DG, P)],
                        wo_sb[:, self.h, ts(self.oc, OCH)],
                        start=(self.h == 0), stop=(self.h == HPG - 1),
                    )
                    self.h += 1
                    nmm -= 1
                    if self.h == HPG:
                        ob = osb_pool.tile([P, OCH], bf16, tag="ob")
                        nc.scalar.activation(ob[:], self.ps[:], Copy)
                        nc.sync.dma_start(
                            out[ts(self.m, P), ts(self.oc, OCH)], ob[:]
                        )
                        self.ps = None

            def finish(self):
                while self.units or self.ps is not None:
                    self.step(HPG)

        # deferred softmax-normalization epilogues: (lps, opst, hpair, c)
        pending = []

        def flush_pending():
            while pending:
                lpst, opst, hpair, c = pending.pop(0)
                for h in hpair:
                    lbs = lb_pool.tile([P, CHUNK], fp32, tag="lbs")
                    nc.vector.reciprocal_approx_fast(lbs[:], lpst[h][:])
                    nc.vector.tensor_mul(AT[:, h, c % 2, :], opst[h][:], lbs[:])

        def attn(c, op_emit):
            # interleave diagonal (pattern-masked) tiles evenly among full
            # tiles: their es chain is one DVE hop longer, so spreading them
            # keeps the chain latency hidden
            fulls = [t for t in range(TKT) if actions.get((c, t)) == "full"]
            diags = [t for t in range(TKT)
                     if (c, t) in actions and actions[(c, t)] != "full"]
            n, nd = len(fulls) + len(diags), len(diags)
            tlist = []
            di = fi = 0
            for i in range(n):
                if di < nd and (i + 1) * nd >= (di + 1) * n:
                    tlist.append(diags[di]); di += 1
                else:
                    tlist.append(fulls[fi]); fi += 1
            nslots = 2 * len(tlist)
            total_steps = op_emit.steps_left()
            done_steps = 0
            slot = 0
            ngrp = (len(tlist) + 1) // 2  # l matmuls per head (pair-summed)
            for hpair in ((0, 1), (2, 3)):
                opst = {
                    h: o_psum.tile([P, CHUNK], fp32, tag="o", name=f"opst{h}") for h in hpair
                }
                lpst = {
                    h: l_psum.tile([P, CHUNK], fp32, tag="l", name=f"lpst{h}") for h in hpair
                }
                es_prev = {h: None for h in hpair}
                for i, t in enumerate(tlist):
                    first, last = (i == 0), (i == len(tlist) - 1)
                    if first:
                        flush_pending()
                    es = {}
                    for h in hpair:
                        sps = s_psum.tile([P, CHUNK], fp32, tag="s")
                        nc.tensor.matmul(
                            sps[:], KT[:, h, ts(t, P)], QT[:, h, c % 2, :],
                            start=True, stop=True,
                        )
                        e = exp_pool.tile([P, CHUNK], bf16, tag="es")
                        nc.scalar.activation(e[:], sps[:], Exp, scale=SCALE)
                        a = actions[(c, t)]
                        if a != "full":
                            nc.vector.tensor_mul(e[:], e[:], pat_sb[:, a, :])
                        es[h] = e
                    # row sums: DVE-sum es pairs, one l matmul per pair
                    # (the all-ones [128,128] stationary broadcasts l to
                    # every partition at no extra cost)
                    for h in hpair:
                        if es_prev[h] is None and not last:
                            es_prev[h] = es[h]
                            continue
                        grp = i // 2
                        if es_prev[h] is not None:
                            rhs = exp_pool.tile([P, CHUNK], bf16, tag="es", name="esum")
                            nc.vector.tensor_add(rhs[:], es_prev[h][:], es[h][:])
                            es_prev[h] = None
                        else:
                            rhs = es[h]
                        nc.tensor.matmul(
                            lpst[h][:], ones_sb[:], rhs[:],
                            start=(grp == 0), stop=(grp == ngrp - 1),
                        )
                    for h in hpair:
                        nc.tensor.matmul(
                            opst[h][:], V[:, t, ts(h, HD)], es[h][:],
                            start=first, stop=last,
                        )
                    # interleave outproj 2-MM steps of the previous chunk
                    want = ((slot + 1) * total_steps + nslots - 1) // nslots
                    while done_steps < want and op_emit.steps_left() > 0:
                        op_emit.step(2)
                        done_steps += 1
                    slot += 1
                pending.append((lpst, opst, hpair, c))

        # ---- main per-chunk pipeline ----
        op_emit = OutprojEmitter([])
        for c in range(TCH):
            if c > 0:
                for g in range(NDG):
                    xk_t[(c, g)] = dma_x(xk_pool, "xk", xkvT_r, c, g)
                for g in range(NDG):
                    xq_t[(c, g)] = dma_x(xq_pool, "xq", xqT_r, c, g)
            proj_qk(c, xk_t, wk_sb, KT, ring=False, douter=(c == 0))
            proj_v(c, xk_t, douter=(c == 0))
            proj_qk(c, xq_t, wq_sb, QT, ring=True, douter=(c == 0))
            attn(c, op_emit)
            op_emit.finish()  # whatever didn't fit in the interleave slots
            op_emit = OutprojEmitter(
                [(c * DG + mm, oc) for mm in range(DG) for oc in range(D // OCH)]
            )
        flush_pending()
        op_emit.finish()

    nc.compile()
    return nc


def _interleave_heads(W):
    """Permute each 128-wide head block of columns: new[2j]=old[j], new[2j+1]=old[64+j]."""
    d, gd = W.shape
    return np.ascontiguousarray(
        W.reshape(d, gd // HD, 2, HALF).transpose(0, 1, 3, 2).reshape(d, gd)
    )


def _rope_tables(cos, sin):
    """cs2[2j]=cs2[2j+1]=cos_j ; ss2[2j]=-sin_j, ss2[2j+1]=+sin_j  (both [128, T])."""
    bf = ml_dtypes.bfloat16
    cosT = np.ascontiguousarray(cos.T)  # [HALF, T]
    sinT = np.ascontiguousarray(sin.T)
    cs2 = np.repeat(cosT, 2, axis=0).astype(bf)
    ss2 = np.stack([-sinT, sinT], axis=1).reshape(HD, -1).astype(bf)
    return cs2, ss2


def _mask_actions(mask):
    """Classify every [CHUNK tq x P tk] score block of the mask.

    Returns ({(c, t): "full" | pattern_idx}, patterns [npat, P, CHUNK] bf16).
    Blocks with no unmasked element are omitted (skipped entirely).
    Patterns are stored transposed ([tk, tq]) to match the score layout.
    """
    m = np.asarray(mask).reshape(T, T).astype(bool)
    actions = {}
    pats = []
    pat_keys = {}
    for c in range(TCH):
        for t in range(TKT):
            blk = m[c * CHUNK : (c + 1) * CHUNK, t * P : (t + 1) * P]
            if not blk.any():
                continue
            if blk.all():
                actions[(c, t)] = "full"
                continue
            bt = np.ascontiguousarray(blk.T)
            key = bt.tobytes()
            if key not in pat_keys:
                pat_keys[key] = len(pats)
                pats.append(bt.astype(ml_dtypes.bfloat16))
            actions[(c, t)] = pat_keys[key]
    if not pats:
        pats.append(np.zeros((P, CHUNK), ml_dtypes.bfloat16))
    return actions, np.ascontiguousarray(np.stack(pats))


def kernel(**inputs):
    global LAST_RESULT
    q_input = np.asarray(inputs["q_input"], dtype=np.float32)
    kv_input = np.asarray(inputs["kv_input"], dtype=np.float32)
    cos = np.asarray(inputs["cos"], dtype=np.float32)
    sin = np.asarray(inputs["sin"], dtype=np.float32)
    Wq = np.asarray(inputs["Wq"], dtype=np.float32)
    Wk = np.asarray(inputs["Wk"], dtype=np.float32)
    Wv = np.asarray(inputs["Wv"], dtype=np.float32)
    Wo = np.asarray(inputs["Wo"], dtype=np.float32)

    actions, pats = _mask_actions(inputs["mask"])
    key = (tuple(sorted(actions.items())), pats.shape[0])
    if key not in _cache:
        _cache[key] = _build_program(actions, int(pats.shape[0]))
    nc = _cache[key]

    bf = ml_dtypes.bfloat16
    cs2, ss2 = _rope_tables(cos, sin)
    xq = [np.ascontiguousarray(q_input[b].T).astype(bf) for b in range(B)]
    xkv = [np.ascontiguousarray(kv_input[b].T).astype(bf) for b in range(B)]
    wq_g = [_interleave_heads(Wq[:, g * GD : (g + 1) * GD]).astype(bf) for g in range(GROUPS)]
    wk_g = [_interleave_heads(Wk[:, g * GD : (g + 1) * GD]).astype(bf) for g in range(GROUPS)]
    wv_g = [np.ascontiguousarray(Wv[:, g * GD : (g + 1) * GD]).astype(bf) for g in range(GROUPS)]
    wo_g = [np.ascontiguousarray(Wo[g * GD : (g + 1) * GD, :]).astype(bf) for g in range(GROUPS)]

    in_maps = []
    for core in range(NCORES):
        b, g = divmod(core, GROUPS)
        in_maps.append({
            "xqT": xq[b],
            "xkvT": xkv[b],
            "wq": wq_g[g],
            "wk": wk_g[g],
            "wv": wv_g[g],
            "wo": wo_g[g],
            "cs2": cs2,
            "ss2": ss2,
            "pat": pats,
        })

    from concourse import bass_utils

    res = bass_utils.run_bass_kernel_spmd(
        nc, in_maps, core_ids=list(range(NCORES)), trace=TRACE
    )
    LAST_RESULT = res
    outs = [np.asarray(r["out"]).astype(np.float32) for r in res.results]
    full = np.stack(
        [sum(outs[b * GROUPS + g] for g in range(GROUPS)) for b in range(B)]
    )
    return np.ascontiguousarray(full.astype(np.float32))


# revision 26
# speedup vs baseline: 1.0066x; 1.0066x over previous
"""Multi-head causal attention with RoPE on 8 Trainium2 NeuronCores.

Sharding: data-parallel over batch (2 groups of 4 cores) x tensor-parallel
over heads (4 heads / 512 cols of Wq/Wk/Wv per core, 512 rows of Wo).
Each core computes its head-group's Q/K/V projections in transposed layout
([head_dim, seq] -- so no on-device transposes are ever needed), applies
RoPE, runs causal softmax attention (scores kept transposed [tk, tq];
row sums via a ones-vector matmul), and emits its partial output
projection.  The host sums the 4 partials per batch element.

v2 pipeline structure (PE-bound; per-chunk software pipeline):
  for c: [dma c+1] proj K/V/Q(c) -> attn(c) interleaved with outproj(c-1)
  - row-sum (l) matmuls for a head pair packed into PE column groups 0/64
  - softmax normalization chain (ACT Reciprocal -> broadcast matmul ->
    copy/mul on DVE) software-pipelined so it never blocks the PE
  - all PSUM evictions on VectorE; ScalarE only does exp / rope-copy / recip
  - outputs stored bf16, summed on host in fp32

Self-contained: shapes/sharding hardcoded for
  q_input/kv_input [2, 2048, 2048], 16 heads x 128 head_dim.
"""

import math

import numpy as np
import ml_dtypes

B, T, D, H = 2, 2048, 2048, 16
HD = 128          # head dim
HALF = HD // 2    # rope half
P = 128           # partitions
CHUNK = 512       # tq / free-dim chunk
NCORES = 8
GROUPS = 4        # head-groups (tensor-parallel degree per batch)
HPG = H // GROUPS # heads per group
GD = HPG * HD     # group width (512)
DT = D // P       # d-tiles (16)
DG = 4            # d-tiles per DMA group
NDG = DT // DG    # DMA groups (4)
TCH = T // CHUNK  # seq chunks (4)
TKT = T // P      # tk tiles (16)
OCH = 512         # outproj free-dim chunk

TRACE = False       # set True before calling kernel() to capture an NTFF trace
LAST_RESULT = None  # BassKernelResults of the last kernel() call
PACK_L = True       # pack the two row-sum matmuls of a head pair via col tiling

_cache = {}


def _build_program(actions, npat):
    """Build the per-core Bass program.

    actions: {(c, t): "full" | pattern_index} for every (tq-chunk, tk-tile)
    score block that has at least one unmasked element.
    """
    from contextlib import ExitStack

    import concourse.mybir as mybir
    import concourse.tile as tile
    from concourse import bacc
    from concourse.bass import ts

    fp32 = mybir.dt.float32
    bf16 = mybir.dt.bfloat16
    Copy = mybir.ActivationFunctionType.Copy
    Exp = mybir.ActivationFunctionType.Exp
    SCALE = 1.0 / math.sqrt(HD)

    nc = bacc.Bacc(
        "TRN2",
        target_bir_lowering=False,
        debug=False,
        enable_asserts=False,
        num_devices=NCORES,
    )

    # host-side pre-permuted to partition-major so every DMA line is a
    # single contiguous multi-KB read per partition
    xqT = nc.dram_tensor("xqT", [P, TCH, DT, CHUNK], bf16, kind="ExternalInput").ap()
    xkvT = nc.dram_tensor("xkvT", [P, TCH, DT, CHUNK], bf16, kind="ExternalInput").ap()
    wq = nc.dram_tensor("wq", [P, DT, GD], bf16, kind="ExternalInput").ap()
    wk = nc.dram_tensor("wk", [P, DT, GD], bf16, kind="ExternalInput").ap()
    wv = nc.dram_tensor("wv", [P, DT, GD], bf16, kind="ExternalInput").ap()
    wo = nc.dram_tensor("wo", [P, HPG, D], bf16, kind="ExternalInput").ap()
    # RoPE in head-dim-interleaved space (host permutes Wq/Wk columns so the
    # rope pair (j, j+64) lands on adjacent partitions (2j, 2j+1); scores are
    # invariant to a common Q/K head-dim permutation):
    #   rope'(x) = x * cs2 + swap_adjacent_pairs(x) * ss2
    # cs2[2j] = cs2[2j+1] = cos_j ; ss2[2j] = -sin_j, ss2[2j+1] = +sin_j
    cs2 = nc.dram_tensor("cs2", [P, T], bf16, kind="ExternalInput").ap()
    ss2 = nc.dram_tensor("ss2", [P, T], bf16, kind="ExternalInput").ap()
    pat = nc.dram_tensor("pat", [npat, P, CHUNK], bf16, kind="ExternalInput").ap()
    out = nc.dram_tensor("out", [T, D], bf16, kind="ExternalOutput").ap()

    xqT_r = xqT
    xkvT_r = xkvT

    with ExitStack() as ctx:
        tc = ctx.enter_context(tile.TileContext(nc))
        const_pool = ctx.enter_context(tc.tile_pool(name="const", bufs=1))
        xk_pool = ctx.enter_context(tc.tile_pool(name="xk", bufs=6))
        xq_pool = ctx.enter_context(tc.tile_pool(name="xq", bufs=6))
        rope_pool = ctx.enter_context(tc.tile_pool(name="rope", bufs=3))
        exp_pool = ctx.enter_context(tc.tile_pool(name="exp", bufs=8))
        osb_pool = ctx.enter_context(tc.tile_pool(name="osb", bufs=3))
        lb_pool = ctx.enter_context(tc.tile_pool(name="lb", bufs=2))
        mm_psum = ctx.enter_context(tc.tile_pool(name="mmps", bufs=2, space="PSUM"))
        s_psum = ctx.enter_context(tc.tile_pool(name="sps", bufs=2, space="PSUM"))
        o_psum = ctx.enter_context(tc.tile_pool(name="ops", bufs=2, space="PSUM"))
        l_psum = ctx.enter_context(tc.tile_pool(name="lps", bufs=2, space="PSUM"))

        # persistent SBUF tensors
        wq_sb = const_pool.tile([P, DT, GD], bf16, tag="wq")
        wk_sb = const_pool.tile([P, DT, GD], bf16, tag="wk")
        wv_sb = const_pool.tile([P, DT, GD], bf16, tag="wv")
        wo_sb = const_pool.tile([P, HPG, D], bf16, tag="wo")
        cs2_sb = const_pool.tile([P, T], bf16, tag="cs2")
        ss2_sb = const_pool.tile([P, T], bf16, tag="ss2")
        pat_sb = const_pool.tile([P, npat, CHUNK], bf16, tag="pat")
        # all-ones [128, 128] stationary: the row-sum matmul then emits l
        # broadcast across every output partition at no extra cost (cost ~ N)
        ones_sb = const_pool.tile([P, P], bf16, tag="ones")
        KT = const_pool.tile([P, HPG, T], bf16, tag="KT")
        V = const_pool.tile([P, TKT, GD], bf16, tag="V")
        QT = const_pool.tile([P, HPG, 2, CHUNK], bf16, tag="QT")  # 2-chunk ring
        AT = const_pool.tile([P, HPG, 2, CHUNK], bf16, tag="AT")  # 2-chunk ring

        nc.vector.memset(ones_sb[:], 1.0)

        def dma_w(dst_sb, src, g):
            nc.sync.dma_start(dst_sb[:, ts(g, DG), :], src[:, ts(g, DG), :])

        def dma_x(pool, tag, src_r, c, g):
            t = pool.tile([P, DG, CHUNK], bf16, tag=tag)
            nc.sync.dma_start(t[:], src_r[:, c, ts(g, DG), :])
            return t

        # ---- prologue DMAs, ordered so the PE can start ~immediately ----
        xk_t = {}
        for g in range(NDG):
            dma_w(wk_sb, wk, g)
            xk_t[(0, g)] = dma_x(xk_pool, "xk", xkvT_r, 0, g)
        nc.sync.dma_start(cs2_sb[:], cs2)
        nc.sync.dma_start(ss2_sb[:], ss2)
        for g in range(NDG):
            dma_w(wv_sb, wv, g)
        for g in range(NDG):
            dma_w(wq_sb, wq, g)
        xq_t = {(0, g): dma_x(xq_pool, "xq", xqT_r, 0, g) for g in range(NDG)}
        nc.sync.dma_start(pat_sb[:], pat.rearrange("j p n -> p j n"))
        for h in range(HPG):
            nc.sync.dma_start(wo_sb[:, h, :], wo[:, h, :])

        SHUF_MASK = [i + 1 - 2 * (i % 2) for i in range(32)]  # [1,0,3,2,...]

        def rope_evict(ps, c, dest_ap):
            # ps: PSUM [P, CHUNK] fp32, partitions = interleaved head_dim
            raw = rope_pool.tile([P, CHUNK], bf16, tag="raw")
            nc.scalar.activation(raw[:], ps[:], Copy)
            rsw = rope_pool.tile([P, CHUNK], bf16, tag="rsw")
            nc.vector.stream_shuffle(rsw[:], raw[:], SHUF_MASK)
            t1 = rope_pool.tile([P, CHUNK], bf16, tag="t1")
            nc.vector.tensor_mul(t1[:], raw[:], cs2_sb[:, ts(c, CHUNK)])
            t2 = rope_pool.tile([P, CHUNK], bf16, tag="t2")
            nc.vector.tensor_mul(t2[:], rsw[:], ss2_sb[:, ts(c, CHUNK)])
            nc.vector.tensor_add(dest_ap, t1[:], t2[:])

        def _proj_psums(douter):
            # d-outer mode holds all four accumulators at once: borrow the
            # (idle at that point) s_psum banks for the upper two
            if douter:
                return [
                    mm_psum.tile([P, CHUNK], fp32, tag="mm", name="pj0"),
                    mm_psum.tile([P, CHUNK], fp32, tag="mm", name="pj1"),
                    s_psum.tile([P, CHUNK], fp32, tag="s", name="pj2"),
                    s_psum.tile([P, CHUNK], fp32, tag="s", name="pj3"),
                ]
            return None

        def proj_qk(c, x_tiles, w_sb, dest, ring, douter=False):
            # douter: issue matmuls d-group-major so the PE starts as soon
            # as the first wk/x d-group DMA lands (chunk-0 warmup)
            pss = _proj_psums(douter)
            order = (
                [(h, d) for d in range(DT) for h in range(HPG)]
                if douter else
                [(h, d) for h in range(HPG) for d in range(DT)]
            )
            for h, d in order:
                if pss is None and d == 0:
                    ps = mm_psum.tile([P, CHUNK], fp32, tag="mm")
                    pss_h = {h: ps}
                if pss is None:
                    ps = pss_h[h]
                else:
                    ps = pss[h]
                nc.tensor.matmul(
                    ps[:], w_sb[:, d, ts(h, HD)], x_tiles[(c, d // DG)][:, d % DG, :],
                    start=(d == 0), stop=(d == DT - 1),
                )
                if d == DT - 1:
                    dest_ap = dest[:, h, c % 2, :] if ring else dest[:, h, ts(c, CHUNK)]
                    rope_evict(ps, c, dest_ap)

        def proj_v(c, x_tiles, douter=False):
            pss = _proj_psums(douter)
            order = (
                [(s, d) for d in range(DT) for s in range(DG)]
                if douter else
                [(s, d) for s in range(DG) for d in range(DT)]
            )
            for s, d in order:
                if pss is None and d == 0:
                    ps = mm_psum.tile([P, GD], fp32, tag="mm")
                    pss_s = {s: ps}
                if pss is None:
                    ps = pss_s[s]
                else:
                    ps = pss[s]
                nc.tensor.matmul(
                    ps[:], x_tiles[(c, d // DG)][:, d % DG, ts(s, P)],
                    wv_sb[:, d, :],
                    start=(d == 0), stop=(d == DT - 1),
                )
                if d == DT - 1:
                    nc.vector.tensor_copy(V[:, c * DG + s, :], ps[:])

        class OutprojEmitter:
            """Emit outproj (m, oc) units in 2-matmul steps so attention
            slots can absorb partial units without over/under-filling."""

            def __init__(self, units):
                self.units = list(units)  # (m, oc) pairs
                self.ps = None
                self.h = 0

            def steps_left(self):
                return (len(self.units) * HPG + (HPG - self.h) % HPG) // 2

            def step(self, nmm=2):
                while nmm > 0:
                    if self.ps is None:
                        if not self.units:
                            return
                        self.m, self.oc = self.units.pop(0)
                        self.ps = mm_psum.tile([P, OCH], fp32, tag="mm", name="opps")
                        self.h = 0
                    c = self.m // DG
                    nc.tensor.matmul(
                        self.ps[:],
                        AT[:, self.h, c % 2, ts(self.m % DG, P)],
                        wo_sb[:, self.h, ts(self.oc, OCH)],
                        start=(self.h == 0), stop=(self.h == HPG - 1),
                    )
                    self.h += 1
                    nmm -= 1
                    if self.h == HPG:
                        ob = osb_pool.tile([P, OCH], bf16, tag="ob")
                        nc.vector.tensor_copy(ob[:], self.ps[:])
                        nc.sync.dma_start(
                            out[ts(self.m, P), ts(self.oc, OCH)], ob[:]
                        )
                        self.ps = None

            def finish(self):
                while self.units or self.ps is not None:
                    self.step(HPG)

        # deferred softmax-normalization epilogues: (lps, opst, hpair, c)
        pending = []

        def flush_pending():
            while pending:
                lpst, opst, hpair, c = pending.pop(0)
                for h in hpair:
                    lbs = lb_pool.tile([P, CHUNK], fp32, tag="lbs")
                    nc.vector.reciprocal_approx_fast(lbs[:], lpst[h][:])
                    nc.vector.tensor_mul(AT[:, h, c % 2, :], opst[h][:], lbs[:])

        def attn(c, op_emit):
            # interleave diagonal (pattern-masked) tiles evenly among full
            # tiles: their es chain is one DVE hop longer, so spreading them
            # keeps the chain latency hidden
            fulls = [t for t in range(TKT) if actions.get((c, t)) == "full"]
            diags = [t for t in range(TKT)
                     if (c, t) in actions and actions[(c, t)] != "full"]
            n, nd = len(fulls) + len(diags), len(diags)
            tlist = []
            di = fi = 0
            for i in range(n):
                if di < nd and (i + 1) * nd >= (di + 1) * n:
                    tlist.append(diags[di]); di += 1
                else:
                    tlist.append(fulls[fi]); fi += 1
            nslots = 2 * len(tlist)
            total_steps = op_emit.steps_left()
            done_steps = 0
            slot = 0
            ngrp = (len(tlist) + 1) // 2  # l matmuls per head (pair-summed)
            for hpair in ((0, 1), (2, 3)):
                opst = {
                    h: o_psum.tile([P, CHUNK], fp32, tag="o", name=f"opst{h}") for h in hpair
                }
                lpst = {
                    h: l_psum.tile([P, CHUNK], fp32, tag="l", name=f"lpst{h}") for h in hpair
                }
                es_prev = {h: None for h in hpair}
                for i, t in enumerate(tlist):
                    first, last = (i == 0), (i == len(tlist) - 1)
                    if first:
                        flush_pending()
                    es = {}
                    for h in hpair:
                        sps = s_psum.tile([P, CHUNK], fp32, tag="s")
                        nc.tensor.matmul(
                            sps[:], KT[:, h, ts(t, P)], QT[:, h, c % 2, :],
                            start=True, stop=True,
                        )
                        e = exp_pool.tile([P, CHUNK], bf16, tag="es")
                        nc.scalar.activation(e[:], sps[:], Exp, scale=SCALE)
                        a = actions[(c, t)]
                        if a != "full":
                            nc.vector.tensor_mul(e[:], e[:], pat_sb[:, a, :])
                        es[h] = e
                    # row sums: DVE-sum es pairs, one l matmul per pair
                    # (the all-ones [128,128] stationary broadcasts l to
                    # every partition at no extra cost)
                    for h in hpair:
                        if es_prev[h] is None and not last:
                            es_prev[h] = es[h]
                            continue
                        grp = i // 2
                        if es_prev[h] is not None:
                            rhs = exp_pool.tile([P, CHUNK], bf16, tag="es", name="esum")
                            nc.vector.tensor_add(rhs[:], es_prev[h][:], es[h][:])
                            es_prev[h] = None
                        else:
                            rhs = es[h]
                        nc.tensor.matmul(
                            lpst[h][:], ones_sb[:], rhs[:],
                            start=(grp == 0), stop=(grp == ngrp - 1),
                        )
                    for h in hpair:
                        nc.tensor.matmul(
                            opst[h][:], V[:, t, ts(h, HD)], es[h][:],
                            start=first, stop=last,
                        )
                    # interleave outproj 2-MM steps of the previous chunk
                    want = ((slot + 1) * total_steps + nslots - 1) // nslots
                    while done_steps < want and op_emit.steps_left() > 0:
                        op_emit.step(2)
                        done_steps += 1
                    slot += 1
                pending.append((lpst, opst, hpair, c))

        # ---- main per-chunk pipeline ----
        op_emit = OutprojEmitter([])
        for c in range(TCH):
            if c > 0:
                for g in range(NDG):
                    xk_t[(c, g)] = dma_x(xk_pool, "xk", xkvT_r, c, g)
                for g in range(NDG):
                    xq_t[(c, g)] = dma_x(xq_pool, "xq", xqT_r, c, g)
            proj_qk(c, xk_t, wk_sb, KT, ring=False, douter=(c == 0))
            proj_v(c, xk_t, douter=(c == 0))
            proj_qk(c, xq_t, wq_sb, QT, ring=True, douter=(c == 0))
            attn(c, op_emit)
            op_emit.finish()  # whatever didn't fit in the interleave slots
            op_emit = OutprojEmitter(
                [(c * DG + mm, oc) for mm in range(DG) for oc in range(D // OCH)]
            )
        flush_pending()
        op_emit.finish()

    nc.compile()
    return nc


def _interleave_heads(W):
    """Permute each 128-wide head block of columns: new[2j]=old[j], new[2j+1]=old[64+j]."""
    d, gd = W.shape
    return np.ascontiguousarray(
        W.reshape(d, gd // HD, 2, HALF).transpose(0, 1, 3, 2).reshape(d, gd)
    )


def _rope_tables(cos, sin):
    """cs2[2j]=cs2[2j+1]=cos_j ; ss2[2j]=-sin_j, ss2[2j+1]=+sin_j  (both [128, T])."""
    bf = ml_dtypes.bfloat16
    cosT = np.ascontiguousarray(cos.T)  # [HALF, T]
    sinT = np.ascontiguousarray(sin.T)
    cs2 = np.repeat(cosT, 2, axis=0).astype(bf)
    ss2 = np.stack([-sinT, sinT], axis=1).reshape(HD, -1).astype(bf)
    return cs2, ss2


def _mask_actions(mask):
    """Classify every [CHUNK tq x P tk] score block of the mask.

    Returns ({(c, t): "full" | pattern_idx}, patterns [npat, P, CHUNK] bf16).
    Blocks with no unmasked element are omitted (skipped entirely).
    Patterns are stored transposed ([tk, tq]) to match the score layout.
    """
    m = np.asarray(mask).reshape(T, T).astype(bool)
    actions = {}
    pats = []
    pat_keys = {}
    for c in range(TCH):
        for t in range(TKT):
            blk = m[c * CHUNK : (c + 1) * CHUNK, t * P : (t + 1) * P]
            if not blk.any():
                continue
            if blk.all():
                actions[(c, t)] = "full"
                continue
            bt = np.ascontiguousarray(blk.T)
            key = bt.tobytes()
            if key not in pat_keys:
                pat_keys[key] = len(pats)
                pats.append(bt.astype(ml_dtypes.bfloat16))
            actions[(c, t)] = pat_keys[key]
    if not pats:
        pats.append(np.zeros((P, CHUNK), ml_dtypes.bfloat16))
    return actions, np.ascontiguousarray(np.stack(pats))


def kernel(**inputs):
    global LAST_RESULT
    q_input = np.asarray(inputs["q_input"], dtype=np.float32)
    kv_input = np.asarray(inputs["kv_input"], dtype=np.float32)
    cos = np.asarray(inputs["cos"], dtype=np.float32)
    sin = np.asarray(inputs["sin"], dtype=np.float32)
    Wq = np.asarray(inputs["Wq"], dtype=np.float32)
    Wk = np.asarray(inputs["Wk"], dtype=np.float32)
    Wv = np.asarray(inputs["Wv"], dtype=np.float32)
    Wo = np.asarray(inputs["Wo"], dtype=np.float32)

    actions, pats = _mask_actions(inputs["mask"])
    key = (tuple(sorted(actions.items())), pats.shape[0])
    if key not in _cache:
        _cache[key] = _build_program(actions, int(pats.shape[0]))
    nc = _cache[key]

    bf = ml_dtypes.bfloat16
    cs2, ss2 = _rope_tables(cos, sin)

    def _x_pm(x):  # [T, D] -> [P, TCH, DT, CHUNK] partition-major
        return np.ascontiguousarray(
            x.T.reshape(DT, P, TCH, CHUNK).transpose(1, 2, 0, 3)
        ).astype(bf)

    def _w_pm(w):  # [D, GD] -> [P, DT, GD]
        return np.ascontiguousarray(
            w.reshape(DT, P, GD).transpose(1, 0, 2)
        ).astype(bf)

    xq = [_x_pm(q_input[b]) for b in range(B)]
    xkv = [_x_pm(kv_input[b]) for b in range(B)]
    wq_g = [_w_pm(_interleave_heads(Wq[:, g * GD : (g + 1) * GD])) for g in range(GROUPS)]
    wk_g = [_w_pm(_interleave_heads(Wk[:, g * GD : (g + 1) * GD])) for g in range(GROUPS)]
    wv_g = [_w_pm(Wv[:, g * GD : (g + 1) * GD]) for g in range(GROUPS)]
    wo_g = [
        np.ascontiguousarray(
            Wo[g * GD : (g + 1) * GD, :].reshape(HPG, P, D).transpose(1, 0, 2)
        ).astype(bf)
        for g in range(GROUPS)
    ]

    in_maps = []
    for core in range(NCORES):
        b, g = divmod(core, GROUPS)
        in_maps.append({
            "xqT": xq[b],
            "xkvT": xkv[b],
            "wq": wq_g[g],
            "wk": wk_g[g],
            "wv": wv_g[g],
            "wo": wo_g[g],
            "cs2": cs2,
            "ss2": ss2,
            "pat": pats,
        })

    from concourse import bass_utils

    res = bass_utils.run_bass_kernel_spmd(
        nc, in_maps, core_ids=list(range(NCORES)), trace=TRACE
    )
    LAST_RESULT = res
    outs = [np.asarray(r["out"]).astype(np.float32) for r in res.results]
    full = np.stack(
        [sum(outs[b * GROUPS + g] for g in range(GROUPS)) for b in range(B)]
    )
    return np.ascontiguousarray(full.astype(np.float32))
